# revision 1
# baseline (speedup 1.0000x reference)
"""Trainium2 Bass kernel for ragged-sequence attention (G2/f16/split-DMA).

Per batch b:
    tq     = tanh(query[b] @ W + bias)                      [CA, H]
    scores = key[b] @ tq.T                                  [S, CA]
    alpha  = exp(scores) * (s < seq_len[b])                 [S, CA]
    out[b] = (alpha.T @ value[b]) / alpha.sum(axis=0)[:,None]

Strategy (HBM-bandwidth bound; everything serves DMA bytes):
  - Raggedness: independent 128-row sub-chunks of each valid prefix;
    numerator/denominator are additive over s, each sub yields a partial
    [CA, 768+1] (col 768 = denominator via a ones-column in the value tile).
  - Subs are spread round-robin over 8 cores, packed 2 per "group"; one
    group = two DMAs (~0.5MB keyT/tq/mask half on the SP HWDGE ring, ~0.4MB
    value half on the ACT HWDGE ring) for parallel descriptor streams.
    Identical NEFF on all cores (SPMD); dummy subs have zero tq/mask.
    Host does the tiny group-by-batch reduction and division.
  - Streams in fp16 (better mantissa than bf16 at equal bytes; inputs are
    O(1)-ranged so fp16's range is ample). BASS_ATTN_DT=bf16 / f32r
    switch the stream dtype. exp and psum accumulation stay fp32;
    partial outputs return as fp16.
  - key is pre-transposed on the host into [128, 6, 128] h-major tiles
    (4-byte dtypes have no DMA-transpose path, and the host does it for
    free); value tiles are [128, 772] s-major with ones at col 768.
"""

import os
import sys

import numpy as np

for _p in ("/opt/trn_rl_repo", "/root/.axon_site/_ro/trn_rl_repo"):
    if os.path.isdir(_p) and _p not in sys.path:
        sys.path.append(_p)

N_CORES = 8
SUB = 128        # rows per work item (= matmul contraction dim)
G = 3            # sub-chunks per group (one DMA / processing slot)
H = 768
HSUB = H // 128  # 6
CA = 32
VW = 772         # value tile width: 768 value cols + ones col @768 + pad
NQ = VW // 4     # 193: value matmul runs as 4 PE col-tiles -> one psum bank

TQ_W = HSUB * CA              # 192 per sub
TQ_OFF = 0
MK_OFF = TQ_OFF + G * TQ_W    # 384
MK_W = G                      # 2
ID_OFF = MK_OFF + MK_W        # 386
ID_W = CA                     # 32
KT_OFF = ID_OFF + ID_W        # 418
KT_W = HSUB * SUB             # 768 per sub
VL_OFF = KT_OFF + G * KT_W    # 1954
COMB_W = VL_OFF + G * VW      # 3498

DT = os.environ.get("BASS_ATTN_DT", "f16")

_module_cache = {}
_last_in_maps = None


def _np_dt():
    if DT == "bf16":
        import ml_dtypes

        return ml_dtypes.bfloat16
    if DT == "f16":
        return np.float16
    return np.float32


def _build_module(nch, loop_r=None):
    import contextlib
    import concourse.mybir as mybir
    import concourse.tile as tile
    from concourse import bacc

    f32 = mybir.dt.float32
    f16 = mybir.dt.float16
    mmdt = {
        "bf16": mybir.dt.bfloat16,
        "f16": mybir.dt.float16,
        "f32r": mybir.dt.float32r,
    }[DT]
    AF = mybir.ActivationFunctionType

    nc = bacc.Bacc(None, target_bir_lowering=False, enable_asserts=False)
    comb_d = nc.dram_tensor("comb", [nch, 128, COMB_W], mmdt, kind="ExternalInput")
    out_d = nc.dram_tensor("outp", [nch, 128, G, NQ], f16, kind="ExternalOutput")

    with tile.TileContext(nc) as tc:
        with (
            tc.tile_pool(name="big", bufs=8) as big,
            tc.tile_pool(name="work", bufs=5) as work,
            tc.tile_pool(name="ps_s", bufs=2, space="PSUM") as ps_s_pool,
            tc.tile_pool(name="ps_t", bufs=2, space="PSUM") as ps_t_pool,
            tc.tile_pool(name="ps_o", bufs=3, space="PSUM") as ps_o_pool,
            tc.For_i(0, loop_r, 1) if loop_r else contextlib.nullcontext(),
        ):
            for i in range(nch):
                ct = big.tile([128, COMB_W], mmdt, tag="comb")
                # kt/tq/mask half on the SP HWDGE ring, value half on the
                # ACT HWDGE ring: parallel descriptor streams
                nc.sync.dma_start(out=ct[:, :VL_OFF], in_=comb_d[i, :, :VL_OFF])
                nc.scalar.dma_start(out=ct[:, VL_OFF:], in_=comb_d[i, :, VL_OFF:])

                tq_v = ct[:, TQ_OFF : TQ_OFF + G * TQ_W].rearrange(
                    "p (m o c) -> p m o c", m=G, o=HSUB
                )
                mk_v = ct[:, MK_OFF : MK_OFF + MK_W]
                id_v = ct[:CA, ID_OFF : ID_OFF + ID_W]
                kt_v = ct[:, KT_OFF : KT_OFF + G * KT_W].rearrange(
                    "p (m o s) -> p m o s", m=G, o=HSUB
                )
                vl_v = ct[:, VL_OFF : VL_OFF + G * VW].rearrange(
                    "p (m w) -> p m w", m=G
                )

                # scores.T: [CA, G*SUB]; sub m -> columns [m*SUB, (m+1)*SUB)
                ps_s = ps_s_pool.tile([CA, G * SUB], f32)
                for m in range(G):
                    for ho in range(HSUB):
                        nc.tensor.matmul(
                            ps_s[:, m * SUB : (m + 1) * SUB],
                            lhsT=tq_v[:, m, ho, :],
                            rhs=kt_v[:, m, ho, :],
                            start=(ho == 0),
                            stop=(ho == HSUB - 1),
                        )

                sb_e = work.tile([CA, G * SUB], mmdt, tag="exp")
                nc.scalar.activation(out=sb_e, in_=ps_s, func=AF.Exp)

                # transpose exp(scores) to s-on-partitions for the value mm
                ps_t = ps_t_pool.tile([128, G, CA], mmdt)
                for m in range(G):
                    nc.tensor.transpose(
                        ps_t[:, m, :],
                        sb_e[:, m * SUB : (m + 1) * SUB],
                        id_v,
                    )

                al_t = work.tile([128, G, CA], mmdt, tag="alpha")
                nc.vector.tensor_tensor(
                    al_t,
                    ps_t,
                    mk_v[:, :, None].to_broadcast([128, G, CA]),
                    mybir.AluOpType.mult,
                )

                # numerator (+ denominator via ones column at 768) per sub:
                # 4 concurrent PE col-tiles land the [CA, VW] output as
                # [128, NQ] in ONE psum bank, so the PSUM->SBUF copy uses
                # all 128 lanes (4x fewer cycles than a [CA, VW] copy)
                ob = work.tile([128, G, NQ], f16, tag="ob")
                for m in range(G):
                    ps_o = ps_o_pool.tile([128, NQ], f32, tag="ps_o")
                    for j in range(4):
                        nc.tensor.matmul(
                            ps_o[32 * j : 32 * (j + 1), :],
                            lhsT=al_t[:, m, :],
                            rhs=vl_v[:, m, NQ * j : NQ * (j + 1)],
                            start=True,
                            stop=True,
                            tile_position=(0, 32 * j),
                        )
                    if m < G - 1:
                        nc.vector.tensor_copy(out=ob[:, m, :], in_=ps_o)
                    else:
                        nc.scalar.copy(out=ob[:, m, :], in_=ps_o)
                nc.sync.dma_start(out=out_d[i], in_=ob)

    nc.compile()
    return nc


def kernel(key, value, query, seq_len, W, b):
    key = np.ascontiguousarray(np.asarray(key, dtype=np.float32))
    value = np.ascontiguousarray(np.asarray(value, dtype=np.float32))
    query = np.asarray(query, dtype=np.float32)
    W = np.asarray(W, dtype=np.float32)
    bias = np.asarray(b, dtype=np.float32)
    sl = np.asarray(seq_len).astype(np.int64)

    B, S, H_ = key.shape
    assert H_ == H and S % SUB == 0

    # host: tiny projection  tq[b] = tanh(query[b] @ W + bias)  [B, CA, H]
    tq = np.tanh(query.reshape(B * query.shape[1], -1) @ W + bias)
    tq = tq.reshape(B, query.shape[1], H).astype(np.float32)
    npdt = _np_dt()
    tqT_p = {
        bi: np.ascontiguousarray(tq[bi].T.reshape(HSUB, 128, CA)).astype(npdt)
        for bi in range(B)
    }

    # work list: 128-row sub-chunks over valid prefixes
    subs = []  # (batch, s0, nvalid)
    for bi in range(B):
        L = int(sl[bi])
        L = max(1, min(L, S))
        for s0 in range(0, L, SUB):
            subs.append((bi, s0, min(SUB, L - s0)))
    total = len(subs)
    per_core = -(-total // N_CORES)
    nch = -(-per_core // G)

    comb = np.zeros((N_CORES, nch, 128, COMB_W), npdt)
    comb[:, :, :CA, ID_OFF : ID_OFF + ID_W] = np.eye(CA, dtype=np.float32)
    slot_map = [[] for _ in range(N_CORES)]  # per core: list of (slot, m, batch)

    for idx, (bi, s0, nval) in enumerate(subs):
        c = idx % N_CORES
        k = idx // N_CORES
        j, m = k // G, k % G
        row = comb[c, j]
        row[:, TQ_OFF + m * TQ_W : TQ_OFF + (m + 1) * TQ_W] = (
            tqT_p[bi].transpose(1, 0, 2).reshape(128, TQ_W)
        )
        mcol = np.zeros(128, np.float32)
        mcol[:nval] = 1.0
        row[:, MK_OFF + m] = mcol
        kc = key[bi, s0 : s0 + SUB]  # [SUB, H]
        row[:, KT_OFF + m * KT_W : KT_OFF + (m + 1) * KT_W] = (
            kc.T.reshape(HSUB, 128, SUB).transpose(1, 0, 2).reshape(128, KT_W)
        )
        vt = row[:, VL_OFF + m * VW : VL_OFF + (m + 1) * VW]
        vt[:, :H] = value[bi, s0 : s0 + SUB]
        vt[:, H] = 1.0
        slot_map[c].append((j, m, bi))

    if nch not in _module_cache:
        _module_cache[nch] = _build_module(nch)
    nc = _module_cache[nch]

    from concourse.bass_utils import run_bass_kernel_spmd

    in_maps = [{"comb": comb[c]} for c in range(N_CORES)]
    global _last_in_maps
    _last_in_maps = in_maps
    trace = os.environ.get("BASS_KERNEL_TRACE") == "1"
    kwargs = {}
    if trace:
        kwargs = dict(trace=True, trace_cores=list(range(N_CORES)))
    res = run_bass_kernel_spmd(nc, in_maps, core_ids=list(range(N_CORES)), **kwargs)
    if trace and res.exec_time_ns is not None:
        print(f"HW exec time: {res.exec_time_ns} ns")
        print(f"HW exec time mean: {res.mean_exec_time_ns} ns")

    num = np.zeros((B, CA, H), np.float64)
    den = np.zeros((B, CA), np.float64)
    for c in range(N_CORES):
        part = res.results[c]["outp"]  # [nch, 128, G, NQ] col-tiled quarters
        for j, m, bi in slot_map[c]:
            blk = part[j, :, m, :].astype(np.float64).reshape(4, CA, NQ)
            full = np.concatenate(list(blk), axis=1)
            num[bi] += full[:, :H]
            den[bi] += full[:, H]
    out = (num / den[:, :, None]).astype(np.float32)
    return out



# revision 37
# speedup vs baseline: 1.3870x; 1.3870x over previous
"""Trainium2 Bass kernel for ragged-sequence attention (fp8 stream, flipped MMs).

Per batch b:
    tq     = tanh(query[b] @ W + bias)                      [CA, H]
    scores = key[b] @ tq.T                                  [S, CA]
    alpha  = exp(scores) * (s < seq_len[b])                 [S, CA]
    out[b] = (alpha.T @ value[b]) / alpha.sum(axis=0)[:,None]

Strategy (HBM-bandwidth bound; everything serves DMA bytes):
  - Work items: 128-row sub-chunks of each valid prefix. Each sub yields an
    independent partial numerator [6*128h x 32c] + denominator [32c];
    the host reduces partials per batch and divides.
  - Streams key (pre-scaled x32) and value in fp8 e3m4 for long batches
    (>=3 subs), fp16 for short batches (<=2 subs, where quantization noise
    doesn't average out). tq stays fp16 (mixed-dtype matmul), alpha fp16.
  - All matmuls are oriented so the *moving* (cost-bearing) dim is CA=32:
    scores_T[s,c] += keyT_tile[h,s]^T @ tqT_tile[h,c]   (6 h-tiles)
    num[h,c]      =  value_tile[s,h]^T @ alpha[s,c]     (6 h-tiles)
    den[1,c]      =  ones[s,1]^T       @ alpha[s,c]
  - Masking + exp shift fused into the activation: alpha = Exp(scores/32 + b)
    with a per-partition bias column (-1.5 valid row, -100 masked/dummy).
  - SPMD-uniform module: all cores run identical slot templates
    (c8 fp8 slots + c16 fp16 slots); slot *data* differs per core, dummies
    are zero-filled (bias -100 -> alpha 0).
  - key is pre-transposed on the host into [128, 6, 128] h-major tiles;
    value stays s-major [128, 768] with a ones column for the denominator.
"""

import os
import sys

import numpy as np

for _p in ("/opt/trn_rl_repo", "/root/.axon_site/_ro/trn_rl_repo"):
    if os.path.isdir(_p) and _p not in sys.path:
        sys.path.append(_p)

N_CORES = 8
SUB = 128
H = 768
HS = H // 128   # 6
CA = 32
KSCALE = 32.0   # key pre-scale so e3m4 sees normal-range values
SHIFT = -1.5    # exp bias shift (cancels in num/den); keeps fp16 partials safe
MASKB = -14.0   # mask bias (e3m4-exact): exp(-14+eps) underflows f16 to ~0
F16_SUB_MAX = 2  # batches with <= this many subs stream in fp16

WTQ = HS * CA + 1              # 193:  tqT 192 | bias 1
WKV = HS * SUB + H + 1         # 1537: keyT 768 | value 768 | ones 1
W8 = WTQ + WKV                 # 1730: all e3m4
W16 = WTQ + WKV                # 1730: all f16
G8 = 6                         # fp8 slots per chunk (one DMA)
G16 = 1                        # f16 slots per chunk

_module_cache = {}
_last_in_maps = None


def _np_f8():
    import ml_dtypes

    return ml_dtypes.float8_e3m4


def _plan(c8, c16):
    """Slot order: one fp8 first (fast fill), f16 slots early (fat DMAs stay
    off the critical tail), fp8 rest. Every slot is its own input DMA."""
    order = []
    if c8:
        order.append("f8")
    order += ["f16"] * c16
    order += ["f8"] * (c8 - 1) if c8 else []
    return order


def _obgroups(n):
    """Output groups of 2 (keeps out-DMA descriptors >=512B)."""
    gs = [2] * (n // 2)
    if n % 2:
        gs.append(1)
    return gs


def _build_module(c8, c16, depth=2):
    import concourse.mybir as mybir
    import concourse.tile as tile
    from concourse import bacc

    f32 = mybir.dt.float32
    f16 = mybir.dt.float16
    f8 = mybir.dt.float8e3
    AF = mybir.ActivationFunctionType

    order = _plan(c8, c16)
    nslots = c8 + c16

    nc = bacc.Bacc(None, target_bir_lowering=False, enable_asserts=False)
    comb8 = (
        nc.dram_tensor("comb8", [128, c8 * W8], f8, kind="ExternalInput")
        if c8
        else None
    )
    comb16 = (
        nc.dram_tensor("comb16", [128, c16 * W16], f16, kind="ExternalInput")
        if c16
        else None
    )
    # per slot: num [128, 192] f16 + den row-0 [1, 32] f16
    out_d = nc.dram_tensor(
        "outp", [128, nslots * (HS + 1) * CA], f16, kind="ExternalOutput"
    )

    groups = _obgroups(nslots)

    with tile.TileContext(nc) as tc:
        with (
            tc.tile_pool(name="big", bufs=8) as big,
            tc.tile_pool(name="al", bufs=6) as al_pool,
            tc.tile_pool(name="ob", bufs=3) as ob_pool,
            tc.tile_pool(name="ps_s", bufs=4, space="PSUM") as ps_s_pool,
            tc.tile_pool(name="ps_n", bufs=4, space="PSUM") as ps_n_pool,
        ):
            # ---- stage the whole slot schedule (tiles + APs) up front ----
            # one input DMA per slot: compute never waits on a fat multi-slot
            # transfer, only on its own 1730B-per-line slice
            slots = []  # per-slot view dicts
            i8 = 0  # fp8-slot ordinal (comb8 offset)
            i16 = 0  # f16-slot ordinal (comb16 offset)
            gidx, m = 0, 0
            ob = None
            for slot, kind in enumerate(order):
                if kind == "f8":
                    ct = big.tile([128, W8], f8, tag="c8", name="ct8")
                    nc.sync.dma_start(
                        out=ct, in_=comb8[:, i8 * W8 : (i8 + 1) * W8]
                    )
                    i8 += 1
                else:
                    ct = big.tile([128, W16], f16, tag="c16", name="ct16")
                    nc.sync.dma_start(
                        out=ct, in_=comb16[:, i16 * W16 : (i16 + 1) * W16]
                    )
                    i16 += 1

                if m == 0:
                    g = groups[gidx]
                    ob = ob_pool.tile(
                        [128, g * (HS + 1) * CA], f16, tag="ob", name="ob"
                    )
                tqb = ct[:, :WTQ]
                base = WTQ
                slots.append(
                    dict(
                        tqv=tqb[:, : HS * CA].rearrange("p (o c) -> p o c", o=HS),
                        bias=tqb[:, HS * CA : HS * CA + 1],
                        ktv=ct[:, base : base + HS * SUB].rearrange(
                            "p (o s) -> p o s", o=HS
                        ),
                        vlv=ct[:, base + HS * SUB : base + HS * SUB + H],
                        ones=ct[:, base + HS * SUB + H : base + WKV],
                        ob=ob,
                        m=m,
                        g=groups[gidx],
                        s0=slot - m,
                        idx=slot,
                    )
                )
                m += 1
                if m == groups[gidx]:
                    gidx += 1
                    m = 0

            # ---- software-pipelined emission ----
            # PE order: scores(i) runs `depth` slots ahead of value(i), so the
            # in-order PE queue never stalls on ACT's exp.
            def emit_scores(s):
                ps_s = ps_s_pool.tile([CA * 4, CA], f32, tag="pss", name="pss")
                s["ps_s"] = ps_s
                for o in range(HS):
                    nc.tensor.matmul(
                        ps_s,
                        lhsT=s["ktv"][:, o, :],
                        rhs=s["tqv"][:, o, :],
                        start=(o == 0),
                        stop=(o == HS - 1),
                    )
                # alpha = Exp(scores/KSCALE + bias); bias also masks
                al = al_pool.tile([128, CA], f16, tag="al", name="al")
                s["al"] = al
                nc.scalar.activation(
                    out=al,
                    in_=s["ps_s"],
                    func=AF.Exp,
                    bias=s["bias"],
                    scale=1.0 / KSCALE,
                )

            def emit_tail(s):
                al = s["al"]
                W = (HS + 1) * CA
                # num[h-tile o] = value_o^T @ alpha -> [128h, 32c];
                # den = ones^T @ alpha -> [1, 32c] lands in the same psum tile
                # at row 0, cols 192..224 (junk in partitions 1-127 there is
                # ignored by the host)
                ps_n = ps_n_pool.tile([128, W], f32, tag="psn", name="psn")
                for o in range(HS):
                    nc.tensor.matmul(
                        ps_n[:, o * CA : (o + 1) * CA],
                        lhsT=s["vlv"][:, o * SUB : (o + 1) * SUB],
                        rhs=al,
                        start=True,
                        stop=True,
                    )
                m, g = s["m"], s["g"]
                nc.tensor.matmul(
                    ps_n[0:1, HS * CA :],
                    lhsT=s["ones"],
                    rhs=al,
                    start=True,
                    stop=True,
                )
                nc.vector.tensor_copy(
                    out=s["ob"][:, m * W : (m + 1) * W], in_=ps_n
                )
                if m == g - 1:
                    s0 = s["s0"]
                    # out-DMA on SP: all input descriptor-gens are emitted
                    # first, so these waits never delay an input
                    nc.sync.dma_start(
                        out=out_d[:, s0 * W : (s0 + g) * W],
                        in_=s["ob"],
                    )

            n = len(slots)
            for i in range(n + depth):
                if i < n:
                    emit_scores(slots[i])
                if i - depth >= 0:
                    emit_tail(slots[i - depth])

    nc.compile()
    return nc


def kernel(key, value, query, seq_len, W, b):
    key = np.ascontiguousarray(np.asarray(key, dtype=np.float32))
    value = np.ascontiguousarray(np.asarray(value, dtype=np.float32))
    query = np.asarray(query, dtype=np.float32)
    W_ = np.asarray(W, dtype=np.float32)
    bias_in = np.asarray(b, dtype=np.float32)
    sl = np.asarray(seq_len).astype(np.int64)

    B, S, H_ = key.shape
    assert H_ == H and S % SUB == 0
    CA_ = query.shape[1]
    assert CA_ == CA

    f8 = _np_f8()

    # host: tiny projection tq[b] = tanh(query[b] @ W + bias) -> [128p, 6o, 32c]
    tq = np.tanh(query.reshape(B * CA, -1) @ W_ + bias_in)
    tq = tq.reshape(B, CA, H).astype(np.float32)
    tqT = {
        bi: np.ascontiguousarray(
            tq[bi].T.reshape(HS, 128, CA).transpose(1, 0, 2)
        ).reshape(128, HS * CA)
        for bi in range(B)
    }

    # work list
    subs8, subs16 = [], []  # (batch, s0, nval)
    for bi in range(B):
        L = int(max(1, min(sl[bi], S)))
        nsub = -(-L // SUB)
        dst = subs16 if nsub <= F16_SUB_MAX else subs8
        for s0 in range(0, L, SUB):
            dst.append((bi, s0, min(SUB, L - s0)))
    n8, n16 = len(subs8), len(subs16)

    # uniform per-core template: prefer moving fp8 leftovers into f16 slots
    # when that shrinks total bytes
    cands = []
    c8a = -(-n8 // N_CORES)
    c16a = -(-n16 // N_CORES)
    cands.append((c8a, c16a))
    c8b = n8 // N_CORES
    c16b = -(-(n16 + (n8 - c8b * N_CORES)) // N_CORES)
    cands.append((c8b, c16b))
    cost = lambda c: c[0] * W8 + c[1] * 2 * W16
    c8, c16 = min(cands, key=cost)
    nslots = c8 + c16

    comb8 = np.zeros((N_CORES, 128, c8 * W8), f8)
    comb16 = np.zeros((N_CORES, 128, c16 * W16), np.float16)
    comb8[:, :, WTQ - 1 :: W8] = MASKB  # dummy default: bias masks everything
    comb16[:, :, WTQ - 1 :: W16] = MASKB
    slot_map = [[] for _ in range(N_CORES)]  # (slot, batch)

    def fill_slot(arr, col0, bi, s0, nval, npdt, kscale):
        # keyT [128p, 6o, 128s]
        kc = np.zeros((SUB, H), np.float32)
        kc[:nval] = key[bi, s0 : s0 + nval] * kscale
        arr[:, col0 : col0 + HS * SUB] = (
            kc.T.reshape(HS, 128, SUB).transpose(1, 0, 2).reshape(128, HS * SUB)
        ).astype(npdt)
        vc = arr[:, col0 + HS * SUB : col0 + HS * SUB + H]
        vc[:nval] = value[bi, s0 : s0 + nval].astype(npdt)
        arr[:, col0 + WKV - 1] = npdt(1.0)

    def fill_tq(arr, col0, bi, nval, npdt):
        arr[:, col0 : col0 + HS * CA] = tqT[bi].astype(npdt)
        bcol = np.full(128, MASKB, np.float32)
        bcol[:nval] = SHIFT
        arr[:, col0 + WTQ - 1] = bcol.astype(npdt)

    # ordinal -> global slot index (slot order) for output decode
    glob8, glob16 = [], []
    for gi, kind in enumerate(_plan(c8, c16)):
        (glob8 if kind == "f8" else glob16).append(gi)

    # deal fp8 subs: first 8*c8 into fp8 slots, leftovers join the f16 pool
    over8 = subs8[N_CORES * c8 :]
    for idx, (bi, s0, nval) in enumerate(subs8[: N_CORES * c8]):
        c, k = idx // c8, idx % c8
        fill_slot(comb8[c], k * W8 + WTQ, bi, s0, nval, f8, KSCALE)
        fill_tq(comb8[c], k * W8, bi, nval, f8)
        slot_map[c].append((glob8[k], bi))
    for idx, (bi, s0, nval) in enumerate(subs16 + over8):
        c, k = idx // c16, idx % c16
        fill_slot(comb16[c], k * W16 + WTQ, bi, s0, nval, np.float16, KSCALE)
        fill_tq(comb16[c], k * W16, bi, nval, np.float16)
        slot_map[c].append((glob16[k], bi))

    ck = (c8, c16)
    if ck not in _module_cache:
        _module_cache[ck] = _build_module(c8, c16)
    nc = _module_cache[ck]

    from concourse.bass_utils import run_bass_kernel_spmd

    in_maps = []
    for c in range(N_CORES):
        m = {}
        if c8:
            m["comb8"] = comb8[c]
        if c16:
            m["comb16"] = comb16[c]
        in_maps.append(m)
    global _last_in_maps
    _last_in_maps = in_maps
    trace = os.environ.get("BASS_KERNEL_TRACE") == "1"
    kwargs = {}
    if trace:
        kwargs = dict(trace=True, trace_cores=list(range(N_CORES)))
    res = run_bass_kernel_spmd(nc, in_maps, core_ids=list(range(N_CORES)), **kwargs)
    if trace and res.exec_time_ns is not None:
        print(f"HW exec time: {res.exec_time_ns} ns")
        print(f"HW exec time mean: {res.mean_exec_time_ns} ns")

    num = np.zeros((B, CA, H), np.float64)
    den = np.zeros((B, CA), np.float64)
    WS = (HS + 1) * CA
    for c in range(N_CORES):
        parts = res.results[c]["outp"]  # [128, nslots*224] f16
        for k, bi in slot_map[c]:
            blk = parts[:, k * WS : k * WS + HS * CA]
            # [128p, 6o, 32c] -> num[b, c, o*128+p]
            num[bi] += (
                blk.astype(np.float64)
                .reshape(128, HS, CA)
                .transpose(2, 1, 0)
                .reshape(CA, H)
            )
            den[bi] += parts[0, k * WS + HS * CA : (k + 1) * WS].astype(np.float64)
    out = (num / den[:, :, None]).astype(np.float32)
    return out


# revision 42
# speedup vs baseline: 1.4111x; 1.0174x over previous
"""Trainium2 Bass kernel for ragged-sequence attention (fp8 stream, flipped MMs).

Per batch b:
    tq     = tanh(query[b] @ W + bias)                      [CA, H]
    scores = key[b] @ tq.T                                  [S, CA]
    alpha  = exp(scores) * (s < seq_len[b])                 [S, CA]
    out[b] = (alpha.T @ value[b]) / alpha.sum(axis=0)[:,None]

Strategy (HBM-bandwidth bound; everything serves DMA bytes):
  - Work items: 128-row sub-chunks of each valid prefix. Each sub yields an
    independent partial numerator [6*128h x 32c] + denominator [32c];
    the host reduces partials per batch and divides.
  - Streams key (pre-scaled x32) and value in fp8 e3m4 for long batches
    (>=3 subs), fp16 for short batches (<=2 subs, where quantization noise
    doesn't average out). tq stays fp16 (mixed-dtype matmul), alpha fp16.
  - All matmuls are oriented so the *moving* (cost-bearing) dim is CA=32:
    scores_T[s,c] += keyT_tile[h,s]^T @ tqT_tile[h,c]   (6 h-tiles)
    num[h,c]      =  value_tile[s,h]^T @ alpha[s,c]     (6 h-tiles)
    den[1,c]      =  ones[s,1]^T       @ alpha[s,c]
  - Masking + exp shift fused into the activation: alpha = Exp(scores/32 + b)
    with a per-partition bias column (-1.5 valid row, -100 masked/dummy).
  - SPMD-uniform module: all cores run identical slot templates
    (c8 fp8 slots + c16 fp16 slots); slot *data* differs per core, dummies
    are zero-filled (bias -100 -> alpha 0).
  - key is pre-transposed on the host into [128, 6, 128] h-major tiles;
    value stays s-major [128, 768] with a ones column for the denominator.
"""

import os
import sys

import numpy as np

for _p in ("/opt/trn_rl_repo", "/root/.axon_site/_ro/trn_rl_repo"):
    if os.path.isdir(_p) and _p not in sys.path:
        sys.path.append(_p)

N_CORES = 8
SUB = 128
H = 768
HS = H // 128   # 6
CA = 32
KSCALE = 32.0   # key pre-scale so e3m4 sees normal-range values
SHIFT = -1.5    # exp bias shift (cancels in num/den); keeps fp16 partials safe
F16_SUB_MAX = 2  # batches with <= this many subs stream in fp16

WTQ = HS * CA                  # 192:  tqT
WKV = HS * SUB + H + 1         # 1537: keyT 768 | value 768 | ones 1
W8 = WTQ + WKV                 # 1730: all e3m4
W16 = WTQ + WKV                # 1730: all f16
G8 = 6                         # fp8 slots per chunk (one DMA)
G16 = 1                        # f16 slots per chunk

_module_cache = {}
_last_in_maps = None


def _np_f8():
    import ml_dtypes

    return ml_dtypes.float8_e3m4


def _plan(c8, c16):
    """Slot order: one fp8 first (fast fill), f16 slots early (fat DMAs stay
    off the critical tail), fp8 rest. Every slot is its own input DMA."""
    order = []
    if c8:
        order.append("f8")
    order += ["f16"] * c16
    order += ["f8"] * (c8 - 1) if c8 else []
    return order


def _obgroups(n):
    """Output groups of 2 (keeps out-DMA descriptors >=512B)."""
    gs = [2] * (n // 2)
    if n % 2:
        gs.append(1)
    return gs


def _build_module(c8, c16, depth=2):
    import concourse.mybir as mybir
    import concourse.tile as tile
    from concourse import bacc

    f32 = mybir.dt.float32
    f16 = mybir.dt.float16
    f8 = mybir.dt.float8e3
    AF = mybir.ActivationFunctionType

    order = _plan(c8, c16)
    nslots = c8 + c16

    nc = bacc.Bacc(None, target_bir_lowering=False, enable_asserts=False)
    comb8 = (
        nc.dram_tensor("comb8", [128, c8 * W8], f8, kind="ExternalInput")
        if c8
        else None
    )
    comb16 = (
        nc.dram_tensor("comb16", [128, c16 * W16], f16, kind="ExternalInput")
        if c16
        else None
    )
    # per slot: num [128, 192] f16 + den row-0 [1, 32] f16
    out_d = nc.dram_tensor(
        "outp", [128, nslots * (HS + 1) * CA], f16, kind="ExternalOutput"
    )

    groups = _obgroups(nslots)

    with tile.TileContext(nc) as tc:
        with (
            tc.tile_pool(name="big", bufs=8) as big,
            tc.tile_pool(name="al", bufs=6) as al_pool,
            tc.tile_pool(name="ob", bufs=3) as ob_pool,
            tc.tile_pool(name="ps_s", bufs=4, space="PSUM") as ps_s_pool,
            tc.tile_pool(name="ps_n", bufs=4, space="PSUM") as ps_n_pool,
        ):
            # shared exp-shift bias column (scalar const, one memset)
            shift_t = nc.alloc_sbuf_tensor("shiftc", [128, 1], f32)
            nc.gpsimd.memset(shift_t.ap(), SHIFT)

            # ---- stage the whole slot schedule (tiles + APs) up front ----
            # one input DMA per slot: compute never waits on a fat multi-slot
            # transfer, only on its own 1730B-per-line slice
            slots = []  # per-slot view dicts
            i8 = 0  # fp8-slot ordinal (comb8 offset)
            i16 = 0  # f16-slot ordinal (comb16 offset)
            gidx, m = 0, 0
            ob = None
            for slot, kind in enumerate(order):
                if kind == "f8":
                    ct = big.tile([128, W8], f8, tag="c8", name="ct8")
                    nc.sync.dma_start(
                        out=ct, in_=comb8[:, i8 * W8 : (i8 + 1) * W8]
                    )
                    i8 += 1
                else:
                    ct = big.tile([128, W16], f16, tag="c16", name="ct16")
                    nc.sync.dma_start(
                        out=ct, in_=comb16[:, i16 * W16 : (i16 + 1) * W16]
                    )
                    i16 += 1

                if m == 0:
                    g = groups[gidx]
                    ob = ob_pool.tile(
                        [128, g * (HS + 1) * CA], f16, tag="ob", name="ob"
                    )
                tqb = ct[:, :WTQ]
                base = WTQ
                slots.append(
                    dict(
                        tqv=tqb.rearrange("p (o c) -> p o c", o=HS),
                        ktv=ct[:, base : base + HS * SUB].rearrange(
                            "p (o s) -> p o s", o=HS
                        ),
                        vlv=ct[:, base + HS * SUB : base + HS * SUB + H],
                        ones=ct[:, base + HS * SUB + H : base + WKV],
                        ob=ob,
                        m=m,
                        g=groups[gidx],
                        s0=slot - m,
                        idx=slot,
                    )
                )
                m += 1
                if m == groups[gidx]:
                    gidx += 1
                    m = 0

            # ---- software-pipelined emission over slot PAIRS ----
            # Masking lives in the ones column (host zeroes masked rows) and
            # zero-padded key/value, so exp uses a scalar bias and one
            # activation serves a whole pair -- half the PE<->ACT round-trips.
            pairs = [slots[j : j + 2] for j in range(0, nslots, 2)]

            def emit_scores(pr):
                k = len(pr)
                ps_s = ps_s_pool.tile([128, k * CA], f32, tag="pss", name="pss")
                for m, s in enumerate(pr):
                    for o in range(HS):
                        nc.tensor.matmul(
                            ps_s[:, m * CA : (m + 1) * CA],
                            lhsT=s["ktv"][:, o, :],
                            rhs=s["tqv"][:, o, :],
                            start=(o == 0),
                            stop=(o == HS - 1),
                        )
                al = al_pool.tile([128, k * CA], f16, tag="al", name="al")
                for m, s in enumerate(pr):
                    s["al"] = al[:, m * CA : (m + 1) * CA]
                nc.scalar.activation(
                    out=al,
                    in_=ps_s,
                    func=AF.Exp,
                    bias=shift_t.ap(),
                    scale=1.0 / KSCALE,
                )

            def emit_tail(pr):
                k = len(pr)
                W = (HS + 1) * CA
                # per slot m: num[h-tile o] = value_o^T @ alpha -> [128h, 32c]
                # at cols m*224+o*32; den = ones^T @ alpha -> [1, 32c] at row 0
                # cols m*224+192 (junk in partitions 1-127 there is ignored)
                ps_n = ps_n_pool.tile([128, k * W], f32, tag="psn", name="psn")
                for m, s in enumerate(pr):
                    al = s["al"]
                    for o in range(HS):
                        nc.tensor.matmul(
                            ps_n[:, m * W + o * CA : m * W + (o + 1) * CA],
                            lhsT=s["vlv"][:, o * SUB : (o + 1) * SUB],
                            rhs=al,
                            start=True,
                            stop=True,
                        )
                    nc.tensor.matmul(
                        ps_n[0:1, m * W + HS * CA : (m + 1) * W],
                        lhsT=s["ones"],
                        rhs=al,
                        start=True,
                        stop=True,
                    )
                s = pr[0]
                nc.vector.tensor_copy(out=s["ob"], in_=ps_n)
                # out-DMA on SP: all input descriptor-gens are emitted
                # first, so these waits never delay an input
                s0 = s["s0"]
                nc.sync.dma_start(
                    out=out_d[:, s0 * W : (s0 + len(pr)) * W],
                    in_=s["ob"],
                )

            n = len(pairs)
            for i in range(n + depth):
                if i < n:
                    emit_scores(pairs[i])
                if i - depth >= 0:
                    emit_tail(pairs[i - depth])

    nc.compile()
    return nc


def kernel(key, value, query, seq_len, W, b):
    key = np.ascontiguousarray(np.asarray(key, dtype=np.float32))
    value = np.ascontiguousarray(np.asarray(value, dtype=np.float32))
    query = np.asarray(query, dtype=np.float32)
    W_ = np.asarray(W, dtype=np.float32)
    bias_in = np.asarray(b, dtype=np.float32)
    sl = np.asarray(seq_len).astype(np.int64)

    B, S, H_ = key.shape
    assert H_ == H and S % SUB == 0
    CA_ = query.shape[1]
    assert CA_ == CA

    f8 = _np_f8()

    # host: tiny projection tq[b] = tanh(query[b] @ W + bias) -> [128p, 6o, 32c]
    tq = np.tanh(query.reshape(B * CA, -1) @ W_ + bias_in)
    tq = tq.reshape(B, CA, H).astype(np.float32)
    tqT = {
        bi: np.ascontiguousarray(
            tq[bi].T.reshape(HS, 128, CA).transpose(1, 0, 2)
        ).reshape(128, HS * CA)
        for bi in range(B)
    }

    # work list
    subs8, subs16 = [], []  # (batch, s0, nval)
    for bi in range(B):
        L = int(max(1, min(sl[bi], S)))
        nsub = -(-L // SUB)
        dst = subs16 if nsub <= F16_SUB_MAX else subs8
        for s0 in range(0, L, SUB):
            dst.append((bi, s0, min(SUB, L - s0)))
    n8, n16 = len(subs8), len(subs16)

    # uniform per-core template: prefer moving fp8 leftovers into f16 slots
    # when that shrinks total bytes
    cands = []
    c8a = -(-n8 // N_CORES)
    c16a = -(-n16 // N_CORES)
    cands.append((c8a, c16a))
    c8b = n8 // N_CORES
    c16b = -(-(n16 + (n8 - c8b * N_CORES)) // N_CORES)
    cands.append((c8b, c16b))
    cost = lambda c: c[0] * W8 + c[1] * 2 * W16
    c8, c16 = min(cands, key=cost)
    nslots = c8 + c16

    comb8 = np.zeros((N_CORES, 128, c8 * W8), f8)
    comb16 = np.zeros((N_CORES, 128, c16 * W16), np.float16)
    slot_map = [[] for _ in range(N_CORES)]  # (slot, batch)

    def fill_slot(arr, col0, bi, s0, nval, npdt, kscale):
        # keyT [128p, 6o, 128s]
        kc = np.zeros((SUB, H), np.float32)
        kc[:nval] = key[bi, s0 : s0 + nval] * kscale
        arr[:, col0 : col0 + HS * SUB] = (
            kc.T.reshape(HS, 128, SUB).transpose(1, 0, 2).reshape(128, HS * SUB)
        ).astype(npdt)
        vc = arr[:, col0 + HS * SUB : col0 + HS * SUB + H]
        vc[:nval] = value[bi, s0 : s0 + nval].astype(npdt)
        # ones column doubles as the ragged mask: den = sum(ones * alpha)
        arr[:nval, col0 + WKV - 1] = npdt(1.0)

    def fill_tq(arr, col0, bi, nval, npdt):
        arr[:, col0 : col0 + HS * CA] = tqT[bi].astype(npdt)

    # ordinal -> global slot index (slot order) for output decode
    glob8, glob16 = [], []
    for gi, kind in enumerate(_plan(c8, c16)):
        (glob8 if kind == "f8" else glob16).append(gi)

    # deal fp8 subs: first 8*c8 into fp8 slots, leftovers join the f16 pool
    over8 = subs8[N_CORES * c8 :]
    for idx, (bi, s0, nval) in enumerate(subs8[: N_CORES * c8]):
        c, k = idx // c8, idx % c8
        fill_slot(comb8[c], k * W8 + WTQ, bi, s0, nval, f8, KSCALE)
        fill_tq(comb8[c], k * W8, bi, nval, f8)
        slot_map[c].append((glob8[k], bi))
    for idx, (bi, s0, nval) in enumerate(subs16 + over8):
        c, k = idx // c16, idx % c16
        fill_slot(comb16[c], k * W16 + WTQ, bi, s0, nval, np.float16, KSCALE)
        fill_tq(comb16[c], k * W16, bi, nval, np.float16)
        slot_map[c].append((glob16[k], bi))

    ck = (c8, c16)
    if ck not in _module_cache:
        _module_cache[ck] = _build_module(c8, c16)
    nc = _module_cache[ck]

    from concourse.bass_utils import run_bass_kernel_spmd

    in_maps = []
    for c in range(N_CORES):
        m = {}
        if c8:
            m["comb8"] = comb8[c]
        if c16:
            m["comb16"] = comb16[c]
        in_maps.append(m)
    global _last_in_maps
    _last_in_maps = in_maps
    trace = os.environ.get("BASS_KERNEL_TRACE") == "1"
    kwargs = {}
    if trace:
        kwargs = dict(trace=True, trace_cores=list(range(N_CORES)))
    res = run_bass_kernel_spmd(nc, in_maps, core_ids=list(range(N_CORES)), **kwargs)
    if trace and res.exec_time_ns is not None:
        print(f"HW exec time: {res.exec_time_ns} ns")
        print(f"HW exec time mean: {res.mean_exec_time_ns} ns")

    num = np.zeros((B, CA, H), np.float64)
    den = np.zeros((B, CA), np.float64)
    WS = (HS + 1) * CA
    for c in range(N_CORES):
        parts = res.results[c]["outp"]  # [128, nslots*224] f16
        for k, bi in slot_map[c]:
            blk = parts[:, k * WS : k * WS + HS * CA]
            # [128p, 6o, 32c] -> num[b, c, o*128+p]
            num[bi] += (
                blk.astype(np.float64)
                .reshape(128, HS, CA)
                .transpose(2, 1, 0)
                .reshape(CA, H)
            )
            den[bi] += parts[0, k * WS + HS * CA : (k + 1) * WS].astype(np.float64)
    out = (num / den[:, :, None]).astype(np.float32)
    return out
